# revision 4
# baseline (speedup 1.0000x reference)
"""Trainium2 Bass kernel for a dense transformer block (pre-LN attention + MLP).

Reference computation (B=4, N=2048, C=1024, H=4096, 16 heads, fp32):
    q = LN(x) @ wq + bq ; k/v = LN(x+pos) @ w{k,v} + b{k,v}
    attn = softmax(q k^T / sqrt(hd)) @ v ; h = x + attn @ wp + bp
    out = h + leaky_relu(LN(h) @ w1 + b1, 0.1) @ w2 + b2

Sharding: 8 cores; core c handles batch c//2, query-token half c%2. K/V
for the full 2048-token sequence are recomputed per core pair (collectives
are slower than the 55us of recompute on this fabric).

v2 design: all matmuls in bf16 (tolerance 2e-2 vs ~1e-3 bf16 error),
weights pre-converted to bf16 host-side (LN gamma folded in), K^T, V and
Q^T live entirely in SBUF in bf16 (no DRAM staging), LayerNorm stats in
fp32. Attention inner loop is software-pipelined so the scores matmul of
tile kt+1 issues before the P@V of tile kt (exp on ACT is the window
bottleneck at ~1038ns/kt vs 853ns/kt of PE work). V carries a ones column
that accumulates the softmax denominator during P@V.
"""

import numpy as np
from contextlib import ExitStack

import concourse.bass as bass
import concourse.bacc as bacc
import concourse.tile as tile
from concourse import mybir
from concourse.masks import make_identity

F32 = mybir.dt.float32
BF16 = mybir.dt.bfloat16
AF = mybir.ActivationFunctionType
ALU = mybir.AluOpType

B, N, C, H, HEADS = 4, 2048, 1024, 4096, 16
HD = C // HEADS            # 64
TQ = N // 2                # query tokens per core = 1024
EPS = 1e-5
SCALE = float(HD) ** -0.5  # 1/8
P = 128
NCORES = 8

NT_KV = N // P             # 16 token tiles (kv side)
NT_Q = TQ // P             # 8 token tiles (q side)
NC_C = C // P              # 8 channel tiles
NO_C = C // 512            # 2 output col tiles of 512
NJ_H = H // P              # 32 H tiles
NRND = 4                   # MLP rounds
JPR = NJ_H // NRND         # 8 H tiles per round


def _ln_stats(nc, pool, x_tile, eps_tile):
    """Return (r, negmr) = (rsqrt(var+eps), -mean*r) as [P,1] fp32 tiles.

    """
    stats = pool.tile([P, 2, 6], F32, tag="bn_stats", name="bn_stats")
    for sg in range(2):
        nc.vector.bn_stats(stats[:, sg, :], x_tile[:, sg * 512:(sg + 1) * 512])
    mv = pool.tile([P, 2], F32, tag="bn_mv", name="bn_mv")
    nc.vector.bn_aggr(mv, stats)
    r = pool.tile([P, 1], F32, tag="ln_r", name="ln_r")
    negmr = pool.tile([P, 1], F32, tag="ln_negmr", name="ln_negmr")
    nc.scalar.activation(r, mv[:, 1:2], AF.Sqrt, bias=eps_tile)
    nc.vector.reciprocal(r, r)
    nc.vector.scalar_tensor_tensor(negmr, mv[:, 0:1], -1.0, r,
                                   ALU.mult, ALU.mult)
    return r, negmr


def build_program():
    nc = bacc.Bacc("TRN2", target_bir_lowering=False, debug=False)

    # Packed I/O: per-launch dispatch cost scales with the number of args
    # (and replicated args are nearly free), so everything rides in three
    # tensors: xs = per-core [xb | xq] (sharded), wr = bf16 weight blob and
    # cr = fp32 const blob (pos + folded biases), both identical on every
    # core (replicated by the host wrapper).
    xs = nc.dram_tensor("xs", [(N + TQ) * C], F32, kind="ExternalInput")
    wr = nc.dram_tensor("wr", [4 * C * C + 2 * C * H], BF16,
                        kind="ExternalInput")
    cr = nc.dram_tensor("cr", [N * C + 5 * C + H], F32, kind="ExternalInput")
    out = nc.dram_tensor("out", [TQ, C], F32, kind="ExternalOutput")

    # blob offsets (element units) — keep in sync with _make_in_maps
    O_XB, O_XQ = 0, N * C
    O_WQ, O_WK, O_WV, O_WP = 0, C * C, 2 * C * C, 3 * C * C
    O_W1, O_W2 = 4 * C * C, 4 * C * C + C * H
    O_POS = 0
    O_CQ, O_CK, O_CV, O_CP = (N * C, N * C + C, N * C + 2 * C, N * C + 3 * C)
    O_C2, O_C1 = N * C + 4 * C, N * C + 5 * C

    def rows(tensor, off, r0, nrows, ncols, rowstride=None):
        """2D [nrows, ncols] view at element offset off + r0*rowstride."""
        rs = ncols if rowstride is None else rowstride
        return bass.AP(tensor=tensor, offset=off + r0 * rs,
                       ap=[[rs, nrows], [1, ncols]])

    xb_t = [rows(xs, O_XB, t * P, P, C) for t in range(NT_KV)]
    pos_t = [rows(cr, O_POS, t * P, P, C) for t in range(NT_KV)]
    xq_t = [rows(xs, O_XQ, t * P, P, C) for t in range(NT_Q)]

    with tile.TileContext(nc) as tc, ExitStack() as ctx:
        const = ctx.enter_context(tc.tile_pool(name="const", bufs=1))
        stat = ctx.enter_context(tc.tile_pool(name="stat", bufs=10))
        ld = ctx.enter_context(tc.tile_pool(name="ld", bufs=2))

        ident_f32 = const.tile([P, P], F32)
        make_identity(nc, ident_f32)
        ident = const.tile([P, P], BF16)
        nc.vector.tensor_copy(ident, ident_f32)
        # eps_tile is produced BY a Sqrt op (eps = sqrt(eps^2)) so the ACT
        # table load happens strictly before the first LayerNorm (every LN
        # Sqrt reads eps_tile) instead of wherever the scheduler drops a
        # free-floating warm-up op.
        eps_sq = const.tile([P, 1], F32)
        nc.vector.memset(eps_sq, EPS * EPS)
        eps_tile = const.tile([P, 1], F32)
        nc.scalar.activation(eps_tile, eps_sq, AF.Sqrt)

        # per-partition bias columns: t[p, j] = blob[off + p + 128*j]
        def col_const(off, n_tiles, name):
            t = const.tile([P, n_tiles], F32, tag=name, name=name)
            nc.sync.dma_start(t, bass.AP(tensor=cr, offset=off,
                                           ap=[[1, P], [P, n_tiles]]))
            return t

        cq_sb = col_const(O_CQ, NC_C, "cq_sb")
        ck_sb = col_const(O_CK, NC_C, "ck_sb")
        c1_sb = col_const(O_C1, NJ_H, "c1_sb")

        # free-dim (row) constants broadcast across all partitions
        def row_const(pool, off, n, name):
            t = pool.tile([P, n], F32, tag=name, name=name)
            nc.sync.dma_start(t, bass.AP(tensor=cr, offset=off,
                                           ap=[[0, P], [1, n]]))
            return t

        # full [C, C] bf16 weight cached in SBUF as [P, NC_C, C] (direct DMA)
        def cache_weight(pool, woff, name):
            wc = pool.tile([P, NC_C, C], BF16, tag=name, name=name)
            for ct in range(NC_C):
                nc.sync.dma_start(wc[:, ct, :], rows(wr, woff, ct * P, P, C))
            return wc

        # normalize + transpose a token tile into xT[:, ct, tcol:tcol+P].
        # Evictions run on the (otherwise idle) Pool engine with a deep psum
        # ring so the PE never stalls waiting for an eviction slot — with
        # ACT-side evictions the 8 transposes paced at ~3.2us/tile and PE
        # idled ~2.8us per tile.
        def norm_transpose(trp, psum_tr, x_tile, xT, tcol):
            r, negmr = _ln_stats(nc, stat, x_tile, eps_tile)
            xn = trp.tile([P, C], BF16, tag="xn", name="xn", bufs=2)
            nc.scalar.activation(xn, x_tile, AF.Identity, bias=negmr, scale=r)
            for ct in range(NC_C):
                ps = psum_tr.tile([P, P], BF16, name="ps_tr")
                nc.tensor.transpose(ps, xn[:, ct * P:(ct + 1) * P], ident)
                nc.scalar.activation(xT[:, ct, tcol:tcol + P], ps, AF.Copy)

        # h tiles live from proj to the end; entered first so the pool stack
        # stays LIFO when kv/at/wp pools release before the MLP phase.
        h_res = ctx.enter_context(tc.tile_pool(name="h_res", bufs=1))
        h_tiles = [h_res.tile([P, C], F32, tag=f"h{i}", name=f"h{i}")
                   for i in range(NT_Q)]

        # K^T, V, Q^T stay resident in SBUF through the attention phase.
        # Manually-scoped pool: freed after proj to make room for the MLP
        # working set.
        kv_cm = tc.tile_pool(name="kv_res", bufs=1)
        kv_res = kv_cm.__enter__()
        K_sb = kv_res.tile([P, NC_C, N], BF16, name="K_sb")       # 32KB/part
        vts = [kv_res.tile([P, HEADS, HD + 1], BF16, tag=f"vt{kt}",
                           name=f"vt{kt}") for kt in range(NT_KV)]  # 33KB/part
        Q_sb = kv_res.tile([P, NC_C, TQ], BF16, name="Q_sb")      # 16KB/part

        # ===== Phase KV + Q: K^T, V for the full sequence; Q fused in =====
        # The Q-side LN/transposes are emitted after the last KV block's
        # matmuls so their DVE/ACT work hides under the K/V PE window and
        # the Q projection (and attention) start with zero PE stall.
        with ExitStack() as front:
            wcache_kv = front.enter_context(tc.tile_pool(name="wcache_kv",
                                                         bufs=1))
            tr_in = front.enter_context(tc.tile_pool(name="tr_in", bufs=2))
            xt_blk = front.enter_context(tc.tile_pool(name="xt_blk", bufs=2))
            psum_mm = front.enter_context(
                tc.tile_pool(name="psum_kv", bufs=4, space="PSUM"))
            psum_tr = front.enter_context(
                tc.tile_pool(name="psum_kvtr", bufs=4, space="PSUM"))

            wk_c = wv_c = wq_c = cv_sb = None

            # PE warm-up: dummy transposes keep the tensor engine busy (and
            # its clock ramping toward full pstate) while the first x/pos
            # tiles and LayerNorm chain trickle through DMA/DVE/ACT. They
            # share the psum_tr ring, so WAW deps keep them ahead of the
            # real transposes.
            for _ in range(50):
                ps = psum_tr.tile([P, P], BF16, name="ps_tr")
                nc.tensor.transpose(ps, ident, ident)

            # 6 pipelined 512-token blocks: 0-3 produce K^T/V of the full
            # sequence (input x+pos), 4-5 produce Q^T (input xq, already
            # resident in the h tiles). Double-buffered xpnT keeps each
            # block's LN/DVE/ACT chain hidden under the previous block's
            # matmuls.
            for blk in range(6):
                xpnT = xt_blk.tile([P, NC_C, 512], BF16, tag="xT",
                                   name="xpnT", bufs=2)
                if blk < 4:
                    # per token tile: LN + transposes immediately followed by
                    # that tile's V matmuls, so the PE chews V work while the
                    # Pool engine drains the next tile's transpose evictions
                    for tt in range(4):
                        t = blk * 4 + tt
                        x_t = ld.tile([P, C], F32, tag="x_in", name="x_in",
                                      bufs=2)
                        nc.sync.dma_start(x_t, xb_t[t])
                        p_t = ld.tile([P, C], F32, tag="p_in", name="p_in",
                                      bufs=2)
                        nc.sync.dma_start(p_t, pos_t[t])
                        xp = tr_in.tile([P, C], F32, tag="xp", name="xp",
                                        bufs=1)
                        nc.vector.tensor_add(xp, x_t, p_t)
                        if blk == 0:
                            # weight loads staggered between the tile loads
                            # on the same (SP) DMA queue: each tile's x/pos
                            # stays ahead of the bulk weight traffic, and no
                            # compute engine's SEQ ever blocks on a DMA
                            if tt == 0:
                                cv_sb = row_const(wcache_kv, O_CV, C,
                                                  "cv_sb")
                                wv_c = cache_weight(wcache_kv, O_WV, "wv_c")
                            elif tt == 1:
                                wk_c = cache_weight(wcache_kv, O_WK, "wk_c")
                            elif tt == 2:
                                wq_c = cache_weight(kv_res, O_WQ, "wq_c")
                        norm_transpose(tr_in, psum_tr, xp, xpnT, tt * P)
                        # ones column of Vtilde (written once per tile)
                        nc.gpsimd.memset(vts[t][:, :, HD:HD + 1], 1.0)
                        for ov in range(NO_C):
                            ps = psum_mm.tile([P, 512], F32, name="ps_mm")
                            for ct in range(NC_C):
                                nc.tensor.matmul(
                                    ps, xpnT[:, ct, tt * P:(tt + 1) * P],
                                    wv_c[:, ct, ov * 512:(ov + 1) * 512],
                                    start=(ct == 0), stop=(ct == NC_C - 1))
                            nc.vector.tensor_add(
                                vts[t][:, ov * 8:(ov + 1) * 8, 0:HD],
                                ps.rearrange("p (h d) -> p h d", d=HD),
                                cv_sb[:, ov * 512:(ov + 1) * 512].rearrange(
                                    "p (h d) -> p h d", d=HD))
                else:
                    for tt in range(4):
                        qt = (blk - 4) * 4 + tt
                        norm_transpose(tr_in, psum_tr, h_tiles[qt], xpnT,
                                       tt * P)
                if blk == 1:
                    # xq rides into the residual tiles: reused for Q-side LN
                    # input and later as the residual base
                    for tt in range(NT_Q):
                        nc.sync.dma_start(h_tiles[tt], xq_t[tt])

                if blk < 4:
                    # K^T[:, this block]  (chan-major)
                    for ot in range(NC_C):
                        ps = psum_mm.tile([P, 512], F32, name="ps_mm")
                        for ct in range(NC_C):
                            nc.tensor.matmul(
                                ps, wk_c[:, ct, ot * P:(ot + 1) * P],
                                xpnT[:, ct, :],
                                start=(ct == 0), stop=(ct == NC_C - 1))
                        nc.scalar.activation(
                            K_sb[:, ot, blk * 512:(blk + 1) * 512], ps,
                            AF.Identity, bias=ck_sb[:, ot:ot + 1])
                else:
                    # Q^T[:, this block]
                    qblk = blk - 4
                    for ot in range(NC_C):
                        ps = psum_mm.tile([P, 512], F32, name="ps_mm")
                        for ct in range(NC_C):
                            nc.tensor.matmul(
                                ps, wq_c[:, ct, ot * P:(ot + 1) * P],
                                xpnT[:, ct, :],
                                start=(ct == 0), stop=(ct == NC_C - 1))
                        nc.vector.tensor_scalar_add(
                            Q_sb[:, ot, qblk * 512:(qblk + 1) * 512], ps,
                            cq_sb[:, ot:ot + 1])

        # attn^T tiles stay resident through proj (manually scoped)
        at_cm = tc.tile_pool(name="at_res", bufs=1)
        at_res = at_cm.__enter__()
        at_tiles = [at_res.tile([P, TQ], BF16, tag=f"at{i}", name=f"at{i}")
                    for i in range(NC_C)]
        wp_cm = tc.tile_pool(name="wcache_p", bufs=1)
        wcache_p = wp_cm.__enter__()
        wp_c = cache_weight(wcache_p, O_WP, "wp_c")
        cp_sb = row_const(wcache_p, O_CP, C, "cp_sb")
        # residual base h = xq + cp loads/adds issued here: the DMA queue and
        # DVE are idle during early attention, keeping this off the
        # attention->proj critical path
        for tt in range(NT_Q):
            nc.sync.dma_start(h_tiles[tt], xq_t[tt])
            nc.vector.tensor_add(h_tiles[tt], h_tiles[tt], cp_sb)

        # ===== Attention =====
        # qb-outer so that during the second query block (qb=1) the finished
        # first-half attn^T columns feed proj matmuls as PE fillers inside
        # the exp-bound inner loop (ACT is the window bottleneck at
        # ~1038ns/kt vs 853ns of PE work). Scores PSUM is bf16, halving its
        # bank footprint to make room for the filler psum pool.
        with ExitStack() as pha:
            pexp = pha.enter_context(tc.tile_pool(name="pexp", bufs=4))
            attn_sc = pha.enter_context(tc.tile_pool(name="attn_sc", bufs=4))
            psum_s = pha.enter_context(
                tc.tile_pool(name="psum_s", bufs=2, space="PSUM"))
            psum_o = pha.enter_context(
                tc.tile_pool(name="psum_o", bufs=2, space="PSUM"))

            for qb in range(TQ // 512):
                for hp in range(HEADS // 2):
                    po = psum_o.tile([HD + 1, 2, 512], F32, name="po")

                    def scores(kt):
                        ps = psum_s.tile([P, 2, 512], F32, name="ps_s")
                        for hh in range(2):
                            o2 = hh * HD
                            nc.tensor.matmul(
                                ps[:, hh, :],
                                K_sb[o2:o2 + HD, hp, kt * P:(kt + 1) * P],
                                Q_sb[o2:o2 + HD, hp,
                                     qb * 512:(qb + 1) * 512],
                                start=True, stop=True)
                        return ps

                    ps_prev = scores(0)
                    for kt in range(NT_KV):
                        pt = pexp.tile([P, 2, 512], BF16, tag="pt",
                                       name="pt")
                        nc.scalar.activation(pt, ps_prev, AF.Exp,
                                             scale=SCALE)
                        if kt + 1 < NT_KV:
                            ps_prev = scores(kt + 1)
                        for hh in range(2):
                            nc.tensor.matmul(
                                po[:, hh, :], vts[kt][:, 2 * hp + hh, :],
                                pt[:, hh, :],
                                start=(kt == 0), stop=(kt == NT_KV - 1))
                    for hh in range(2):
                        o2 = hh * HD
                        recip = attn_sc.tile([1, 512], F32, tag="recip",
                                             name="recip")
                        nc.vector.reciprocal(recip, po[HD:HD + 1, hh, :])
                        rb = attn_sc.tile([HD, 512], F32, tag="rb",
                                          name="rb", bufs=3)
                        nc.gpsimd.partition_broadcast(rb, recip)
                        nc.vector.tensor_mul(
                            at_tiles[hp][o2:o2 + HD,
                                         qb * 512:(qb + 1) * 512],
                            po[0:HD, hh, :], rb)

        # ===== Proj + residual -> h =====
        # LN stats for the MLP are emitted here per-tile so the DVE/ACT work
        # overlaps the proj matmuls; the MLP phase only runs xn + transposes.
        h_stats = []
        with ExitStack() as php:
            psum_mm = php.enter_context(
                tc.tile_pool(name="psum_p", bufs=4, space="PSUM"))
            for tt in range(NT_Q):
                for ov in range(NO_C):
                    ps = psum_mm.tile([P, 512], F32, name="ps_mm")
                    for ct in range(NC_C):
                        nc.tensor.matmul(
                            ps, at_tiles[ct][:, tt * P:(tt + 1) * P],
                            wp_c[:, ct, ov * 512:(ov + 1) * 512],
                            start=(ct == 0), stop=(ct == NC_C - 1))
                    sl = slice(ov * 512, (ov + 1) * 512)
                    nc.vector.tensor_add(h_tiles[tt][:, sl],
                                         h_tiles[tt][:, sl], ps)
                h_stats.append(_ln_stats(nc, stat, h_tiles[tt], eps_tile))

        # wp, attn tiles, kv residents freed after proj (LIFO order)
        wp_cm.__exit__(None, None, None)
        at_cm.__exit__(None, None, None)
        kv_cm.__exit__(None, None, None)

        # ===== MLP: 4 rounds of 8 H-tiles, y2 accumulated into h =====
        with ExitStack() as phm:
            hnt_res = phm.enter_context(tc.tile_pool(name="hnt_res", bufs=1))
            rc_m = phm.enter_context(tc.tile_pool(name="rc_m", bufs=1))
            c2_sb = row_const(rc_m, O_C2, C, "c2_sb")
            tr_in = phm.enter_context(tc.tile_pool(name="tr_in3", bufs=2))
            w1ld = phm.enter_context(tc.tile_pool(name="w1ld", bufs=2))
            w2ld = phm.enter_context(tc.tile_pool(name="w2ld", bufs=2))
            a1p = phm.enter_context(tc.tile_pool(name="a1p", bufs=2))
            mlp_u = phm.enter_context(tc.tile_pool(name="mlp_u", bufs=3))
            psum_m = phm.enter_context(
                tc.tile_pool(name="psum_m", bufs=2, space="PSUM"))
            psum_y = phm.enter_context(
                tc.tile_pool(name="psum_y", bufs=2, space="PSUM"))
            psum_tr = phm.enter_context(
                tc.tile_pool(name="psum_mtr", bufs=4, space="PSUM"))

            def load_w1(rnd):
                w1g = w1ld.tile([P, NC_C, JPR * P], BF16, tag="w1g",
                                name="w1g")
                for ct in range(NC_C):
                    src = bass.AP(tensor=wr,
                                  offset=O_W1 + ct * P * H + rnd * JPR * P,
                                  ap=[[H, P], [1, JPR * P]])
                    nc.sync.dma_start(w1g[:, ct, :], src)
                return w1g

            def load_w2(rnd):
                w2g = w2ld.tile([P, JPR, C], BF16, tag="w2g", name="w2g")
                for j in range(JPR):
                    jt = rnd * JPR + j
                    nc.sync.dma_start(w2g[:, j, :],
                                        rows(wr, O_W2, jt * P, P, C))
                return w2g

            w1g = load_w1(0)
            w2g = load_w2(0)

            hnT = hnt_res.tile([P, NC_C, TQ], BF16, name="hnT")
            for tt in range(NT_Q):
                r, negmr = h_stats[tt]
                xn = tr_in.tile([P, C], BF16, tag="xn", name="xn", bufs=3)
                nc.scalar.activation(xn, h_tiles[tt], AF.Identity,
                                     bias=negmr, scale=r)
                for ct in range(NC_C):
                    ps = psum_tr.tile([P, P], BF16, name="ps_tr")
                    nc.tensor.transpose(ps, xn[:, ct * P:(ct + 1) * P], ident)
                    nc.scalar.activation(hnT[:, ct, tt * P:(tt + 1) * P],
                                         ps, AF.Copy)
            # h becomes the output accumulator: h += c2
            for tt in range(NT_Q):
                nc.vector.tensor_add(h_tiles[tt], h_tiles[tt], c2_sb)

            for rnd in range(NRND):
                a1g = a1p.tile([P, JPR, TQ], BF16, tag="a1g", name="a1g")
                for j in range(JPR):
                    jt = rnd * JPR + j
                    for th in range(2):
                        ps = psum_m.tile([P, 512], F32, name="ps_m")
                        for ct in range(NC_C):
                            nc.tensor.matmul(
                                ps, w1g[:, ct, j * P:(j + 1) * P],
                                hnT[:, ct, th * 512:(th + 1) * 512],
                                start=(ct == 0), stop=(ct == NC_C - 1))
                        # u = y + c1 ; a1 = max(0.1*u, u)  (LeakyReLU 0.1)
                        u = mlp_u.tile([P, 512], F32, tag="u", name="u")
                        nc.scalar.activation(u, ps, AF.Identity,
                                             bias=c1_sb[:, jt:jt + 1])
                        nc.vector.scalar_tensor_tensor(
                            a1g[:, j, th * 512:(th + 1) * 512],
                            u, 0.1, u, ALU.mult, ALU.max)
                # prefetch next round's weights
                if rnd + 1 < NRND:
                    w1g_n = load_w1(rnd + 1)
                    w2g_n = load_w2(rnd + 1)
                for tt in range(NT_Q):
                    for ov in range(NO_C):
                        ps2 = psum_y.tile([P, 512], F32, name="py2")
                        for j in range(JPR):
                            nc.tensor.matmul(
                                ps2, a1g[:, j, tt * P:(tt + 1) * P],
                                w2g[:, j, ov * 512:(ov + 1) * 512],
                                start=(j == 0), stop=(j == JPR - 1))
                        sl = slice(ov * 512, (ov + 1) * 512)
                        nc.vector.tensor_add(h_tiles[tt][:, sl],
                                             h_tiles[tt][:, sl], ps2)
                    if rnd == NRND - 1:
                        # store each finished tile eagerly to overlap the
                        # output DMA with the remaining fc2 compute
                        nc.sync.dma_start(out.ap()[tt * P:(tt + 1) * P, :],
                                          h_tiles[tt])
                if rnd + 1 < NRND:
                    w1g, w2g = w1g_n, w2g_n

    nc.compile()
    return nc


_CACHE = {}


def _get_program():
    if "nc" not in _CACHE:
        _CACHE["nc"] = build_program()
    return _CACHE["nc"]


def _get_exec():
    """Compile once; return (jitted sharded fn, metadata)."""
    if "exec" in _CACHE:
        return _CACHE["exec"]
    import jax
    from jax.experimental.shard_map import shard_map
    from jax.sharding import Mesh, PartitionSpec
    from concourse import bass2jax, mybir as mb

    nc = _get_program()
    bass2jax.install_neuronx_cc_hook()
    partition_name = (nc.partition_id_tensor.name
                      if nc.partition_id_tensor else None)
    in_names, out_names, out_avals, zero_outs = [], [], [], []
    for alloc in nc.m.functions[0].allocations:
        if not isinstance(alloc, mb.MemoryLocationSet):
            continue
        name = alloc.memorylocations[0].name
        if alloc.kind == "ExternalInput":
            if name != partition_name:
                in_names.append(name)
        elif alloc.kind == "ExternalOutput":
            shape = tuple(alloc.tensor_shape)
            dtype = mb.dt.np(alloc.dtype)
            out_names.append(name)
            out_avals.append(jax.core.ShapedArray(shape, dtype))
            zero_outs.append(np.zeros(shape, dtype))
    n_params = len(in_names)
    all_names = list(in_names) + list(out_names)
    if partition_name is not None:
        all_names.append(partition_name)

    def _body(*args):
        operands = list(args)
        if partition_name is not None:
            operands.append(bass2jax.partition_id_tensor())
        outs = bass2jax._bass_exec_p.bind(
            *operands,
            out_avals=tuple(out_avals),
            in_names=tuple(all_names),
            out_names=tuple(out_names),
            lowering_input_output_aliases=(),
            sim_require_finite=True,
            sim_require_nnan=True,
            nc=nc,
        )
        return tuple(outs)

    devices = jax.devices()[:NCORES]
    mesh = Mesh(np.asarray(devices), ("core",))
    # wr/cr are identical across cores: replicate instead of sharding (a
    # replicated operand costs almost nothing per launch on this runtime).
    in_specs = tuple(PartitionSpec() if nm in REPLICATED
                     else PartitionSpec("core") for nm in in_names)
    in_specs = in_specs + (PartitionSpec("core"),) * len(out_names)
    sharded = jax.jit(
        shard_map(_body, mesh=mesh,
                  in_specs=in_specs,
                  out_specs=(PartitionSpec("core"),) * len(out_names),
                  check_rep=False),
        keep_unused=True,
    )
    _CACHE["exec"] = (sharded, mesh, in_names, n_params, out_names,
                      out_avals, zero_outs)
    return _CACHE["exec"]


REPLICATED = {"wr", "cr"}


def _host_args(in_maps):
    """Concatenate sharded args across cores; replicated args pass through."""
    _, _, in_names, _, _, _, zero_outs = _get_exec()
    args = []
    for nm in in_names:
        if nm in REPLICATED:
            args.append(np.asarray(in_maps[0][nm]))
        else:
            args.append(np.concatenate(
                [np.asarray(in_maps[c][nm]) for c in range(NCORES)], axis=0))
    args += [np.zeros((NCORES * z.shape[0], *z.shape[1:]), z.dtype)
             for z in zero_outs]
    return args


def _run(in_maps):
    import jax
    sharded, mesh, in_names, n_params, out_names, out_avals, zero_outs = \
        _get_exec()
    out_arrs = sharded(*_host_args(in_maps))
    jax.block_until_ready(out_arrs)
    return [
        {nm: np.asarray(out_arrs[i]).reshape(NCORES, *out_avals[i].shape)[c]
         for i, nm in enumerate(out_names)}
        for c in range(NCORES)
    ]


def _device_args(in_maps):
    import jax
    from jax.sharding import NamedSharding, PartitionSpec
    sharded, mesh, in_names, n_params, out_names, out_avals, zero_outs = \
        _get_exec()
    sh = NamedSharding(mesh, PartitionSpec("core"))
    shr = NamedSharding(mesh, PartitionSpec())
    host = _host_args(in_maps)
    args = []
    for nm, arr in zip(list(in_names) + ["__out__"] * len(zero_outs), host):
        args.append(jax.device_put(arr, shr if nm in REPLICATED else sh))
    return args


def time_kernel(inputs, iters=5):
    """Marginal per-execute wall time of the compiled executable using
    pipelined async launches: (t(60) - t(10)) / 50, in ns."""
    import time as _time
    import jax
    in_maps = _make_in_maps(**inputs)
    sharded = _get_exec()[0]
    args = _device_args(in_maps)
    jax.block_until_ready(sharded(*args))  # warm

    def run_n(n):
        best = float("inf")
        for _ in range(iters):
            t0 = _time.perf_counter()
            outs = None
            for _i in range(n):
                outs = sharded(*args)
            jax.block_until_ready(outs)
            best = min(best, _time.perf_counter() - t0)
        return best

    t10, t60 = run_n(10), run_n(60)
    return (t60 - t10) / 50.0 * 1e9


def _make_in_maps(x, pos_embed, nq_g, nq_b, nk_g, nk_b, nv_g, nv_b, wq, bq,
                  wk, bk, wv, bv, wp, bp, n_g, n_b, w1, b1, w2, b2):
    import ml_dtypes
    BF = ml_dtypes.bfloat16
    x = np.asarray(x, np.float32)
    pos = np.asarray(pos_embed, np.float32).reshape(N, C)

    def fold(g, b, w, bias):
        w = np.asarray(w, np.float32)
        ws = (np.asarray(g, np.float32)[:, None] * w).astype(BF)
        cst = (np.asarray(b, np.float32) @ w + np.asarray(bias, np.float32))
        return ws, cst

    wq_s, cq_v = fold(nq_g, nq_b, wq, bq)
    wk_s, ck_v = fold(nk_g, nk_b, wk, bk)
    wv_s, cv_v = fold(nv_g, nv_b, wv, bv)
    w1_s, c1_v = fold(n_g, n_b, w1, b1)
    wp_f = np.asarray(wp, np.float32).astype(BF)
    w2_f = np.asarray(w2, np.float32).astype(BF)
    cp_v = np.asarray(bp, np.float32)
    c2_v = np.asarray(b2, np.float32)

    # replicated blobs (order must match the kernel's O_* offsets)
    wr = np.concatenate([w.reshape(-1) for w in
                         (wq_s, wk_s, wv_s, wp_f, w1_s, w2_f)])
    cr = np.concatenate([pos.reshape(-1), cq_v, ck_v, cv_v, cp_v, c2_v,
                         c1_v]).astype(np.float32)

    in_maps = []
    for c in range(NCORES):
        b, half = divmod(c, 2)
        xs = np.concatenate([
            x[b].reshape(-1),
            x[b, half * TQ:(half + 1) * TQ].reshape(-1)])
        in_maps.append({"xs": xs, "wr": wr, "cr": cr})
    return in_maps


def kernel(**inputs):
    results = _run(_make_in_maps(**inputs))
    outa = np.empty((B, N, C), np.float32)
    for c in range(NCORES):
        b, half = divmod(c, 2)
        outa[b, half * TQ:(half + 1) * TQ] = results[c]["out"]
    return outa


# revision 5
# speedup vs baseline: 1.0631x; 1.0631x over previous
"""Trainium2 Bass kernel for a dense transformer block (pre-LN attention + MLP).

Reference computation (B=4, N=2048, C=1024, H=4096, 16 heads, fp32):
    q = LN(x) @ wq + bq ; k/v = LN(x+pos) @ w{k,v} + b{k,v}
    attn = softmax(q k^T / sqrt(hd)) @ v ; h = x + attn @ wp + bp
    out = h + leaky_relu(LN(h) @ w1 + b1, 0.1) @ w2 + b2

Sharding: 8 cores; core c handles batch c//2, query-token half c%2. K/V
for the full 2048-token sequence are recomputed per core pair (collectives
are slower than the 55us of recompute on this fabric).

v2 design: all matmuls in bf16 (tolerance 2e-2 vs ~1e-3 bf16 error),
weights pre-converted to bf16 host-side (LN gamma folded in), K^T, V and
Q^T live entirely in SBUF in bf16 (no DRAM staging), LayerNorm stats in
fp32. Attention inner loop is software-pipelined so the scores matmul of
tile kt+1 issues before the P@V of tile kt (exp on ACT is the window
bottleneck at ~1038ns/kt vs 853ns/kt of PE work). V carries a ones column
that accumulates the softmax denominator during P@V.
"""

import numpy as np
from contextlib import ExitStack

import concourse.bass as bass
import concourse.bacc as bacc
import concourse.tile as tile
from concourse import mybir
from concourse.masks import make_identity

F32 = mybir.dt.float32
BF16 = mybir.dt.bfloat16
AF = mybir.ActivationFunctionType
ALU = mybir.AluOpType

B, N, C, H, HEADS = 4, 2048, 1024, 4096, 16
HD = C // HEADS            # 64
TQ = N // 2                # query tokens per core = 1024
EPS = 1e-5
SCALE = float(HD) ** -0.5  # 1/8
P = 128
NCORES = 8

NT_KV = N // P             # 16 token tiles (kv side)
NT_Q = TQ // P             # 8 token tiles (q side)
NC_C = C // P              # 8 channel tiles
NO_C = C // 512            # 2 output col tiles of 512
NJ_H = H // P              # 32 H tiles
NRND = 4                   # MLP rounds
JPR = NJ_H // NRND         # 8 H tiles per round


def _ln_stats(nc, pool, x_tile, eps_tile):
    """Return (r, negmr) = (rsqrt(var+eps), -mean*r) as [P,1] fp32 tiles.

    """
    stats = pool.tile([P, 2, 6], F32, tag="bn_stats", name="bn_stats")
    for sg in range(2):
        nc.vector.bn_stats(stats[:, sg, :], x_tile[:, sg * 512:(sg + 1) * 512])
    mv = pool.tile([P, 2], F32, tag="bn_mv", name="bn_mv")
    nc.vector.bn_aggr(mv, stats)
    r = pool.tile([P, 1], F32, tag="ln_r", name="ln_r")
    negmr = pool.tile([P, 1], F32, tag="ln_negmr", name="ln_negmr")
    nc.scalar.activation(r, mv[:, 1:2], AF.Sqrt, bias=eps_tile)
    nc.vector.reciprocal(r, r)
    nc.vector.scalar_tensor_tensor(negmr, mv[:, 0:1], -1.0, r,
                                   ALU.mult, ALU.mult)
    return r, negmr


def build_program():
    nc = bacc.Bacc("TRN2", target_bir_lowering=False, debug=False)

    # Packed I/O: per-launch dispatch cost scales with the number of args
    # (and replicated args are nearly free), so everything rides in three
    # tensors: xs = per-core [xb | xq] (sharded), wr = bf16 weight blob and
    # cr = fp32 const blob (pos + folded biases), both identical on every
    # core (replicated by the host wrapper).
    xs = nc.dram_tensor("xs", [(N + TQ) * C], F32, kind="ExternalInput")
    wr = nc.dram_tensor("wr", [4 * C * C + 2 * C * H], BF16,
                        kind="ExternalInput")
    cr = nc.dram_tensor("cr", [N * C + 5 * C + H], F32, kind="ExternalInput")
    out = nc.dram_tensor("out", [TQ, C], F32, kind="ExternalOutput")

    # blob offsets (element units) — keep in sync with _make_in_maps
    O_XB, O_XQ = 0, N * C
    O_WQ, O_WK, O_WV, O_WP = 0, C * C, 2 * C * C, 3 * C * C
    O_W1, O_W2 = 4 * C * C, 4 * C * C + C * H
    O_POS = 0
    O_CQ, O_CK, O_CV, O_CP = (N * C, N * C + C, N * C + 2 * C, N * C + 3 * C)
    O_C2, O_C1 = N * C + 4 * C, N * C + 5 * C

    def rows(tensor, off, r0, nrows, ncols, rowstride=None):
        """2D [nrows, ncols] view at element offset off + r0*rowstride."""
        rs = ncols if rowstride is None else rowstride
        return bass.AP(tensor=tensor, offset=off + r0 * rs,
                       ap=[[rs, nrows], [1, ncols]])

    xb_t = [rows(xs, O_XB, t * P, P, C) for t in range(NT_KV)]
    pos_t = [rows(cr, O_POS, t * P, P, C) for t in range(NT_KV)]
    xq_t = [rows(xs, O_XQ, t * P, P, C) for t in range(NT_Q)]

    with tile.TileContext(nc) as tc, ExitStack() as ctx:
        const = ctx.enter_context(tc.tile_pool(name="const", bufs=1))
        stat = ctx.enter_context(tc.tile_pool(name="stat", bufs=10))
        ld = ctx.enter_context(tc.tile_pool(name="ld", bufs=2))

        ident_f32 = const.tile([P, P], F32)
        make_identity(nc, ident_f32)
        ident = const.tile([P, P], BF16)
        nc.vector.tensor_copy(ident, ident_f32)
        # eps_tile is produced BY a Sqrt op (eps = sqrt(eps^2)) so the ACT
        # table load happens strictly before the first LayerNorm (every LN
        # Sqrt reads eps_tile) instead of wherever the scheduler drops a
        # free-floating warm-up op.
        eps_sq = const.tile([P, 1], F32)
        nc.vector.memset(eps_sq, EPS * EPS)
        eps_tile = const.tile([P, 1], F32)
        nc.scalar.activation(eps_tile, eps_sq, AF.Sqrt)

        # per-partition bias columns: t[p, j] = blob[off + p + 128*j]
        def col_const(off, n_tiles, name):
            t = const.tile([P, n_tiles], F32, tag=name, name=name)
            nc.sync.dma_start(t, bass.AP(tensor=cr, offset=off,
                                           ap=[[1, P], [P, n_tiles]]))
            return t

        cq_sb = col_const(O_CQ, NC_C, "cq_sb")
        ck_sb = col_const(O_CK, NC_C, "ck_sb")
        c1_sb = col_const(O_C1, NJ_H, "c1_sb")

        # free-dim (row) constants broadcast across all partitions
        def row_const(pool, off, n, name):
            t = pool.tile([P, n], F32, tag=name, name=name)
            nc.sync.dma_start(t, bass.AP(tensor=cr, offset=off,
                                           ap=[[0, P], [1, n]]))
            return t

        # full [C, C] bf16 weight cached in SBUF as [P, NC_C, C] (direct DMA)
        def cache_weight(pool, woff, name):
            wc = pool.tile([P, NC_C, C], BF16, tag=name, name=name)
            for ct in range(NC_C):
                nc.sync.dma_start(wc[:, ct, :], rows(wr, woff, ct * P, P, C))
            return wc

        # normalize + transpose a token tile into xT[:, ct, tcol:tcol+P].
        # Evictions run on DVE (PSUM reads are legal there, unlike Pool,
        # and it has slack) so the ACT engine only produces xn — with
        # ACT-side evictions the per-tile ACT chain was ~3.2us and paced
        # the PE transposes.
        def norm_transpose(trp, psum_tr, x_tile, xT, tcol):
            r, negmr = _ln_stats(nc, stat, x_tile, eps_tile)
            xn = trp.tile([P, C], BF16, tag="xn", name="xn", bufs=2)
            nc.scalar.activation(xn, x_tile, AF.Identity, bias=negmr, scale=r)
            for ct in range(NC_C):
                ps = psum_tr.tile([P, P], BF16, name="ps_tr")
                nc.tensor.transpose(ps, xn[:, ct * P:(ct + 1) * P], ident)
                nc.vector.tensor_copy(xT[:, ct, tcol:tcol + P], ps)

        # h tiles live from proj to the end; entered first so the pool stack
        # stays LIFO when kv/at/wp pools release before the MLP phase.
        h_res = ctx.enter_context(tc.tile_pool(name="h_res", bufs=1))
        h_tiles = [h_res.tile([P, C], F32, tag=f"h{i}", name=f"h{i}")
                   for i in range(NT_Q)]

        # K^T, V, Q^T stay resident in SBUF through the attention phase.
        # Manually-scoped pool: freed after proj to make room for the MLP
        # working set.
        kv_cm = tc.tile_pool(name="kv_res", bufs=1)
        kv_res = kv_cm.__enter__()
        K_sb = kv_res.tile([P, NC_C, N], BF16, name="K_sb")       # 32KB/part
        vts = [kv_res.tile([P, HEADS, HD + 1], BF16, tag=f"vt{kt}",
                           name=f"vt{kt}") for kt in range(NT_KV)]  # 33KB/part
        Q_sb = kv_res.tile([P, NC_C, TQ], BF16, name="Q_sb")      # 16KB/part

        # ===== Phase KV + Q: K^T, V for the full sequence; Q fused in =====
        # The Q-side LN/transposes are emitted after the last KV block's
        # matmuls so their DVE/ACT work hides under the K/V PE window and
        # the Q projection (and attention) start with zero PE stall.
        with ExitStack() as front:
            wcache_kv = front.enter_context(tc.tile_pool(name="wcache_kv",
                                                         bufs=1))
            tr_in = front.enter_context(tc.tile_pool(name="tr_in", bufs=2))
            xt_blk = front.enter_context(tc.tile_pool(name="xt_blk", bufs=2))
            psum_mm = front.enter_context(
                tc.tile_pool(name="psum_kv", bufs=4, space="PSUM"))
            psum_tr = front.enter_context(
                tc.tile_pool(name="psum_kvtr", bufs=4, space="PSUM"))

            wk_c = wv_c = wq_c = cv_sb = None

            # PE warm-up: dummy transposes keep the tensor engine busy (and
            # its clock ramping toward full pstate) while the first x/pos
            # tiles and LayerNorm chain trickle through DMA/DVE/ACT. They
            # share the psum_tr ring, so WAW deps keep them ahead of the
            # real transposes.
            for _ in range(50):
                ps = psum_tr.tile([P, P], BF16, name="ps_tr")
                nc.tensor.transpose(ps, ident, ident)

            # 6 pipelined 512-token blocks: 0-3 produce K^T/V of the full
            # sequence (input x+pos), 4-5 produce Q^T (input xq, already
            # resident in the h tiles). Double-buffered xpnT keeps each
            # block's LN/DVE/ACT chain hidden under the previous block's
            # matmuls.
            for blk in range(6):
                xpnT = xt_blk.tile([P, NC_C, 512], BF16, tag="xT",
                                   name="xpnT", bufs=2)
                if blk < 4:
                    # per token tile: LN + transposes immediately followed by
                    # that tile's V matmuls, so the PE chews V work while the
                    # Pool engine drains the next tile's transpose evictions
                    for tt in range(4):
                        t = blk * 4 + tt
                        x_t = ld.tile([P, C], F32, tag="x_in", name="x_in",
                                      bufs=2)
                        nc.sync.dma_start(x_t, xb_t[t])
                        p_t = ld.tile([P, C], F32, tag="p_in", name="p_in",
                                      bufs=2)
                        nc.sync.dma_start(p_t, pos_t[t])
                        xp = tr_in.tile([P, C], F32, tag="xp", name="xp",
                                        bufs=1)
                        nc.vector.tensor_add(xp, x_t, p_t)
                        if blk == 0:
                            # weight loads staggered between the tile loads
                            # on the same (SP) DMA queue: each tile's x/pos
                            # stays ahead of the bulk weight traffic, and no
                            # compute engine's SEQ ever blocks on a DMA
                            if tt == 0:
                                cv_sb = row_const(wcache_kv, O_CV, C,
                                                  "cv_sb")
                                wv_c = cache_weight(wcache_kv, O_WV, "wv_c")
                            elif tt == 1:
                                wk_c = cache_weight(wcache_kv, O_WK, "wk_c")
                            elif tt == 2:
                                wq_c = cache_weight(kv_res, O_WQ, "wq_c")
                        norm_transpose(tr_in, psum_tr, xp, xpnT, tt * P)
                        # ones column of Vtilde (written once per tile)
                        nc.gpsimd.memset(vts[t][:, :, HD:HD + 1], 1.0)
                        for ov in range(NO_C):
                            ps = psum_mm.tile([P, 512], F32, name="ps_mm")
                            for ct in range(NC_C):
                                nc.tensor.matmul(
                                    ps, xpnT[:, ct, tt * P:(tt + 1) * P],
                                    wv_c[:, ct, ov * 512:(ov + 1) * 512],
                                    start=(ct == 0), stop=(ct == NC_C - 1))
                            nc.vector.tensor_add(
                                vts[t][:, ov * 8:(ov + 1) * 8, 0:HD],
                                ps.rearrange("p (h d) -> p h d", d=HD),
                                cv_sb[:, ov * 512:(ov + 1) * 512].rearrange(
                                    "p (h d) -> p h d", d=HD))
                else:
                    for tt in range(4):
                        qt = (blk - 4) * 4 + tt
                        norm_transpose(tr_in, psum_tr, h_tiles[qt], xpnT,
                                       tt * P)
                if blk == 1:
                    # xq rides into the residual tiles: reused for Q-side LN
                    # input and later as the residual base
                    for tt in range(NT_Q):
                        nc.sync.dma_start(h_tiles[tt], xq_t[tt])

                if blk < 4:
                    # K^T[:, this block]  (chan-major)
                    for ot in range(NC_C):
                        ps = psum_mm.tile([P, 512], F32, name="ps_mm")
                        for ct in range(NC_C):
                            nc.tensor.matmul(
                                ps, wk_c[:, ct, ot * P:(ot + 1) * P],
                                xpnT[:, ct, :],
                                start=(ct == 0), stop=(ct == NC_C - 1))
                        nc.scalar.activation(
                            K_sb[:, ot, blk * 512:(blk + 1) * 512], ps,
                            AF.Identity, bias=ck_sb[:, ot:ot + 1])
                else:
                    # Q^T[:, this block]
                    qblk = blk - 4
                    for ot in range(NC_C):
                        ps = psum_mm.tile([P, 512], F32, name="ps_mm")
                        for ct in range(NC_C):
                            nc.tensor.matmul(
                                ps, wq_c[:, ct, ot * P:(ot + 1) * P],
                                xpnT[:, ct, :],
                                start=(ct == 0), stop=(ct == NC_C - 1))
                        nc.vector.tensor_scalar_add(
                            Q_sb[:, ot, qblk * 512:(qblk + 1) * 512], ps,
                            cq_sb[:, ot:ot + 1])

        # attn^T tiles stay resident through proj (manually scoped)
        at_cm = tc.tile_pool(name="at_res", bufs=1)
        at_res = at_cm.__enter__()
        at_tiles = [at_res.tile([P, TQ], BF16, tag=f"at{i}", name=f"at{i}")
                    for i in range(NC_C)]
        wp_cm = tc.tile_pool(name="wcache_p", bufs=1)
        wcache_p = wp_cm.__enter__()
        wp_c = cache_weight(wcache_p, O_WP, "wp_c")
        cp_sb = row_const(wcache_p, O_CP, C, "cp_sb")
        # residual base h = xq + cp loads/adds issued here: the DMA queue and
        # DVE are idle during early attention, keeping this off the
        # attention->proj critical path
        for tt in range(NT_Q):
            nc.sync.dma_start(h_tiles[tt], xq_t[tt])
            nc.vector.tensor_add(h_tiles[tt], h_tiles[tt], cp_sb)

        # ===== Attention =====
        # qb-outer so that during the second query block (qb=1) the finished
        # first-half attn^T columns feed proj matmuls as PE fillers inside
        # the exp-bound inner loop (ACT is the window bottleneck at
        # ~1038ns/kt vs 853ns of PE work). Scores PSUM is bf16, halving its
        # bank footprint to make room for the filler psum pool.
        with ExitStack() as pha:
            pexp = pha.enter_context(tc.tile_pool(name="pexp", bufs=4))
            attn_sc = pha.enter_context(tc.tile_pool(name="attn_sc", bufs=4))
            psum_s = pha.enter_context(
                tc.tile_pool(name="psum_s", bufs=2, space="PSUM"))
            psum_o = pha.enter_context(
                tc.tile_pool(name="psum_o", bufs=2, space="PSUM"))

            for qb in range(TQ // 512):
                for hp in range(HEADS // 2):
                    po = psum_o.tile([HD + 1, 2, 512], F32, name="po")

                    def scores(kt):
                        ps = psum_s.tile([P, 2, 512], F32, name="ps_s")
                        for hh in range(2):
                            o2 = hh * HD
                            nc.tensor.matmul(
                                ps[:, hh, :],
                                K_sb[o2:o2 + HD, hp, kt * P:(kt + 1) * P],
                                Q_sb[o2:o2 + HD, hp,
                                     qb * 512:(qb + 1) * 512],
                                start=True, stop=True)
                        return ps

                    ps_prev = scores(0)
                    for kt in range(NT_KV):
                        pt = pexp.tile([P, 2, 512], BF16, tag="pt",
                                       name="pt")
                        nc.scalar.activation(pt, ps_prev, AF.Exp,
                                             scale=SCALE)
                        if kt + 1 < NT_KV:
                            ps_prev = scores(kt + 1)
                        for hh in range(2):
                            nc.tensor.matmul(
                                po[:, hh, :], vts[kt][:, 2 * hp + hh, :],
                                pt[:, hh, :],
                                start=(kt == 0), stop=(kt == NT_KV - 1))
                    for hh in range(2):
                        o2 = hh * HD
                        recip = attn_sc.tile([1, 512], F32, tag="recip",
                                             name="recip")
                        nc.vector.reciprocal(recip, po[HD:HD + 1, hh, :])
                        rb = attn_sc.tile([HD, 512], F32, tag="rb",
                                          name="rb", bufs=3)
                        nc.gpsimd.partition_broadcast(rb, recip)
                        nc.vector.tensor_mul(
                            at_tiles[hp][o2:o2 + HD,
                                         qb * 512:(qb + 1) * 512],
                            po[0:HD, hh, :], rb)

        # ===== Proj + residual -> h =====
        # LN stats for the MLP are emitted here per-tile so the DVE/ACT work
        # overlaps the proj matmuls; the MLP phase only runs xn + transposes.
        h_stats = []
        with ExitStack() as php:
            psum_mm = php.enter_context(
                tc.tile_pool(name="psum_p", bufs=4, space="PSUM"))
            for tt in range(NT_Q):
                for ov in range(NO_C):
                    ps = psum_mm.tile([P, 512], F32, name="ps_mm")
                    for ct in range(NC_C):
                        nc.tensor.matmul(
                            ps, at_tiles[ct][:, tt * P:(tt + 1) * P],
                            wp_c[:, ct, ov * 512:(ov + 1) * 512],
                            start=(ct == 0), stop=(ct == NC_C - 1))
                    sl = slice(ov * 512, (ov + 1) * 512)
                    nc.vector.tensor_add(h_tiles[tt][:, sl],
                                         h_tiles[tt][:, sl], ps)
                h_stats.append(_ln_stats(nc, stat, h_tiles[tt], eps_tile))

        # wp, attn tiles, kv residents freed after proj (LIFO order)
        wp_cm.__exit__(None, None, None)
        at_cm.__exit__(None, None, None)
        kv_cm.__exit__(None, None, None)

        # ===== MLP: 4 rounds of 8 H-tiles, y2 accumulated into h =====
        with ExitStack() as phm:
            hnt_res = phm.enter_context(tc.tile_pool(name="hnt_res", bufs=1))
            rc_m = phm.enter_context(tc.tile_pool(name="rc_m", bufs=1))
            c2_sb = row_const(rc_m, O_C2, C, "c2_sb")
            tr_in = phm.enter_context(tc.tile_pool(name="tr_in3", bufs=2))
            w1ld = phm.enter_context(tc.tile_pool(name="w1ld", bufs=2))
            w2ld = phm.enter_context(tc.tile_pool(name="w2ld", bufs=2))
            a1p = phm.enter_context(tc.tile_pool(name="a1p", bufs=2))
            mlp_u = phm.enter_context(tc.tile_pool(name="mlp_u", bufs=3))
            psum_m = phm.enter_context(
                tc.tile_pool(name="psum_m", bufs=2, space="PSUM"))
            psum_y = phm.enter_context(
                tc.tile_pool(name="psum_y", bufs=2, space="PSUM"))
            psum_tr = phm.enter_context(
                tc.tile_pool(name="psum_mtr", bufs=4, space="PSUM"))

            def load_w1(rnd):
                w1g = w1ld.tile([P, NC_C, JPR * P], BF16, tag="w1g",
                                name="w1g")
                for ct in range(NC_C):
                    src = bass.AP(tensor=wr,
                                  offset=O_W1 + ct * P * H + rnd * JPR * P,
                                  ap=[[H, P], [1, JPR * P]])
                    nc.sync.dma_start(w1g[:, ct, :], src)
                return w1g

            def load_w2(rnd):
                w2g = w2ld.tile([P, JPR, C], BF16, tag="w2g", name="w2g")
                for j in range(JPR):
                    jt = rnd * JPR + j
                    nc.sync.dma_start(w2g[:, j, :],
                                        rows(wr, O_W2, jt * P, P, C))
                return w2g

            w1g = load_w1(0)
            w2g = load_w2(0)

            hnT = hnt_res.tile([P, NC_C, TQ], BF16, name="hnT")
            for tt in range(NT_Q):
                r, negmr = h_stats[tt]
                xn = tr_in.tile([P, C], BF16, tag="xn", name="xn", bufs=3)
                nc.scalar.activation(xn, h_tiles[tt], AF.Identity,
                                     bias=negmr, scale=r)
                for ct in range(NC_C):
                    ps = psum_tr.tile([P, P], BF16, name="ps_tr")
                    nc.tensor.transpose(ps, xn[:, ct * P:(ct + 1) * P], ident)
                    nc.vector.tensor_copy(hnT[:, ct, tt * P:(tt + 1) * P], ps)
            # h becomes the output accumulator: h += c2
            for tt in range(NT_Q):
                nc.vector.tensor_add(h_tiles[tt], h_tiles[tt], c2_sb)

            for rnd in range(NRND):
                a1g = a1p.tile([P, JPR, TQ], BF16, tag="a1g", name="a1g")
                for j in range(JPR):
                    jt = rnd * JPR + j
                    for th in range(2):
                        ps = psum_m.tile([P, 512], F32, name="ps_m")
                        for ct in range(NC_C):
                            nc.tensor.matmul(
                                ps, w1g[:, ct, j * P:(j + 1) * P],
                                hnT[:, ct, th * 512:(th + 1) * 512],
                                start=(ct == 0), stop=(ct == NC_C - 1))
                        # u = y + c1 ; a1 = max(0.1*u, u)  (LeakyReLU 0.1)
                        u = mlp_u.tile([P, 512], F32, tag="u", name="u")
                        nc.scalar.activation(u, ps, AF.Identity,
                                             bias=c1_sb[:, jt:jt + 1])
                        nc.vector.scalar_tensor_tensor(
                            a1g[:, j, th * 512:(th + 1) * 512],
                            u, 0.1, u, ALU.mult, ALU.max)
                # prefetch next round's weights
                if rnd + 1 < NRND:
                    w1g_n = load_w1(rnd + 1)
                    w2g_n = load_w2(rnd + 1)
                for tt in range(NT_Q):
                    for ov in range(NO_C):
                        ps2 = psum_y.tile([P, 512], F32, name="py2")
                        for j in range(JPR):
                            nc.tensor.matmul(
                                ps2, a1g[:, j, tt * P:(tt + 1) * P],
                                w2g[:, j, ov * 512:(ov + 1) * 512],
                                start=(j == 0), stop=(j == JPR - 1))
                        sl = slice(ov * 512, (ov + 1) * 512)
                        nc.vector.tensor_add(h_tiles[tt][:, sl],
                                             h_tiles[tt][:, sl], ps2)
                    if rnd == NRND - 1:
                        # store each finished tile eagerly to overlap the
                        # output DMA with the remaining fc2 compute
                        nc.sync.dma_start(out.ap()[tt * P:(tt + 1) * P, :],
                                          h_tiles[tt])
                if rnd + 1 < NRND:
                    w1g, w2g = w1g_n, w2g_n

    nc.compile()
    return nc


_CACHE = {}


def _get_program():
    if "nc" not in _CACHE:
        _CACHE["nc"] = build_program()
    return _CACHE["nc"]


def _get_exec():
    """Compile once; return (jitted sharded fn, metadata)."""
    if "exec" in _CACHE:
        return _CACHE["exec"]
    import jax
    from jax.experimental.shard_map import shard_map
    from jax.sharding import Mesh, PartitionSpec
    from concourse import bass2jax, mybir as mb

    nc = _get_program()
    bass2jax.install_neuronx_cc_hook()
    partition_name = (nc.partition_id_tensor.name
                      if nc.partition_id_tensor else None)
    in_names, out_names, out_avals, zero_outs = [], [], [], []
    for alloc in nc.m.functions[0].allocations:
        if not isinstance(alloc, mb.MemoryLocationSet):
            continue
        name = alloc.memorylocations[0].name
        if alloc.kind == "ExternalInput":
            if name != partition_name:
                in_names.append(name)
        elif alloc.kind == "ExternalOutput":
            shape = tuple(alloc.tensor_shape)
            dtype = mb.dt.np(alloc.dtype)
            out_names.append(name)
            out_avals.append(jax.core.ShapedArray(shape, dtype))
            zero_outs.append(np.zeros(shape, dtype))
    n_params = len(in_names)
    all_names = list(in_names) + list(out_names)
    if partition_name is not None:
        all_names.append(partition_name)

    def _body(*args):
        operands = list(args)
        if partition_name is not None:
            operands.append(bass2jax.partition_id_tensor())
        outs = bass2jax._bass_exec_p.bind(
            *operands,
            out_avals=tuple(out_avals),
            in_names=tuple(all_names),
            out_names=tuple(out_names),
            lowering_input_output_aliases=(),
            sim_require_finite=True,
            sim_require_nnan=True,
            nc=nc,
        )
        return tuple(outs)

    devices = jax.devices()[:NCORES]
    mesh = Mesh(np.asarray(devices), ("core",))
    # wr/cr are identical across cores: replicate instead of sharding (a
    # replicated operand costs almost nothing per launch on this runtime).
    in_specs = tuple(PartitionSpec() if nm in REPLICATED
                     else PartitionSpec("core") for nm in in_names)
    in_specs = in_specs + (PartitionSpec("core"),) * len(out_names)
    sharded = jax.jit(
        shard_map(_body, mesh=mesh,
                  in_specs=in_specs,
                  out_specs=(PartitionSpec("core"),) * len(out_names),
                  check_rep=False),
        keep_unused=True,
    )
    _CACHE["exec"] = (sharded, mesh, in_names, n_params, out_names,
                      out_avals, zero_outs)
    return _CACHE["exec"]


REPLICATED = {"wr", "cr"}


def _host_args(in_maps):
    """Concatenate sharded args across cores; replicated args pass through."""
    _, _, in_names, _, _, _, zero_outs = _get_exec()
    args = []
    for nm in in_names:
        if nm in REPLICATED:
            args.append(np.asarray(in_maps[0][nm]))
        else:
            args.append(np.concatenate(
                [np.asarray(in_maps[c][nm]) for c in range(NCORES)], axis=0))
    args += [np.zeros((NCORES * z.shape[0], *z.shape[1:]), z.dtype)
             for z in zero_outs]
    return args


def _run(in_maps):
    import jax
    sharded, mesh, in_names, n_params, out_names, out_avals, zero_outs = \
        _get_exec()
    out_arrs = sharded(*_host_args(in_maps))
    jax.block_until_ready(out_arrs)
    return [
        {nm: np.asarray(out_arrs[i]).reshape(NCORES, *out_avals[i].shape)[c]
         for i, nm in enumerate(out_names)}
        for c in range(NCORES)
    ]


def _device_args(in_maps):
    import jax
    from jax.sharding import NamedSharding, PartitionSpec
    sharded, mesh, in_names, n_params, out_names, out_avals, zero_outs = \
        _get_exec()
    sh = NamedSharding(mesh, PartitionSpec("core"))
    shr = NamedSharding(mesh, PartitionSpec())
    host = _host_args(in_maps)
    args = []
    for nm, arr in zip(list(in_names) + ["__out__"] * len(zero_outs), host):
        args.append(jax.device_put(arr, shr if nm in REPLICATED else sh))
    return args


def time_kernel(inputs, iters=5):
    """Marginal per-execute wall time of the compiled executable using
    pipelined async launches: (t(60) - t(10)) / 50, in ns."""
    import time as _time
    import jax
    in_maps = _make_in_maps(**inputs)
    sharded = _get_exec()[0]
    args = _device_args(in_maps)
    jax.block_until_ready(sharded(*args))  # warm

    def run_n(n):
        best = float("inf")
        for _ in range(iters):
            t0 = _time.perf_counter()
            outs = None
            for _i in range(n):
                outs = sharded(*args)
            jax.block_until_ready(outs)
            best = min(best, _time.perf_counter() - t0)
        return best

    t10, t60 = run_n(10), run_n(60)
    return (t60 - t10) / 50.0 * 1e9


def _make_in_maps(x, pos_embed, nq_g, nq_b, nk_g, nk_b, nv_g, nv_b, wq, bq,
                  wk, bk, wv, bv, wp, bp, n_g, n_b, w1, b1, w2, b2):
    import ml_dtypes
    BF = ml_dtypes.bfloat16
    x = np.asarray(x, np.float32)
    pos = np.asarray(pos_embed, np.float32).reshape(N, C)

    def fold(g, b, w, bias):
        w = np.asarray(w, np.float32)
        ws = (np.asarray(g, np.float32)[:, None] * w).astype(BF)
        cst = (np.asarray(b, np.float32) @ w + np.asarray(bias, np.float32))
        return ws, cst

    wq_s, cq_v = fold(nq_g, nq_b, wq, bq)
    wk_s, ck_v = fold(nk_g, nk_b, wk, bk)
    wv_s, cv_v = fold(nv_g, nv_b, wv, bv)
    w1_s, c1_v = fold(n_g, n_b, w1, b1)
    wp_f = np.asarray(wp, np.float32).astype(BF)
    w2_f = np.asarray(w2, np.float32).astype(BF)
    cp_v = np.asarray(bp, np.float32)
    c2_v = np.asarray(b2, np.float32)

    # replicated blobs (order must match the kernel's O_* offsets)
    wr = np.concatenate([w.reshape(-1) for w in
                         (wq_s, wk_s, wv_s, wp_f, w1_s, w2_f)])
    cr = np.concatenate([pos.reshape(-1), cq_v, ck_v, cv_v, cp_v, c2_v,
                         c1_v]).astype(np.float32)

    in_maps = []
    for c in range(NCORES):
        b, half = divmod(c, 2)
        xs = np.concatenate([
            x[b].reshape(-1),
            x[b, half * TQ:(half + 1) * TQ].reshape(-1)])
        in_maps.append({"xs": xs, "wr": wr, "cr": cr})
    return in_maps


def kernel(**inputs):
    results = _run(_make_in_maps(**inputs))
    outa = np.empty((B, N, C), np.float32)
    for c in range(NCORES):
        b, half = divmod(c, 2)
        outa[b, half * TQ:(half + 1) * TQ] = results[c]["out"]
    return outa
